# revision 13
# baseline (speedup 1.0000x reference)
"""Trainium2 Bass kernel for nn_DGODE (graph ODE over utterance nodes).

Self-contained: hardcodes all shapes. Strategy (v2):
- Row-shard B=4096 nodes over 8 cores (512 rows each). The adjacency is
  effectively banded (exp(-0.1|i-j|) decay): band half-width 128.
- One AllGather per RK4 step boundary (4 total incl. the h0 exchange)
  instead of one per ODE eval: each core redundantly computes k1..k4 on a
  shrinking halo (+-384, +-256, +-128, 0), so no mid-step communication.
- All matmuls in fp16 (f32 PSUM accumulate); dense PE schedule to hold
  the high pstate; b2 bias applied as a rank-1 matmul on the PE.
- S built banded in T-form per (out-group, j-chunk) tile on DVE/gpsimd;
  the static exp(-beta|i-j|) band profile (index-only) is a host table,
  with the diagonal correction folded in (1/A1 trick); row-normalization
  folded into the final fp16 cast.
"""

import sys

if "/opt/trn_rl_repo" not in sys.path:
    sys.path.insert(0, "/opt/trn_rl_repo")

import numpy as np

import concourse.bacc as bacc
import concourse.bass as bass
import concourse.mybir as mybir
import concourse.tile as tile
from concourse.bass_utils import run_bass_kernel_spmd

F32 = mybir.dt.float32
F32R = mybir.dt.float32r
F16 = mybir.dt.float16
U32 = mybir.dt.uint32
AF = mybir.ActivationFunctionType
ALU = mybir.AluOpType

NCORES = 8
B = 4096
D_IN = 1856
D_PAD = 1920           # 15 * 128
ND = D_PAD // 128
H = 128
R = B // NCORES        # 512 rows per core
P = 128
N_STEPS = 4
DT = 1.0 / N_STEPS
A1, A2, BETA = 0.8, 0.5, 0.1

X = 1280               # out-col space: own +-384 (x=0 -> global own_start-384)
NY = 1536              # y space: own +-512, 12 chunks
NCH = NY // P          # 12 y-chunks
GROUPS = [(0, 512), (512, 1024), (1024, 1280)]
JSETS = [list(range(0, 6)), list(range(4, 10)), list(range(8, 12))]

EV_LO = [0, 128, 256, 384]
EV_HI = [1280, 1152, 1024, 896]
YCOEF = [0.5 * DT, 0.5 * DT, DT]        # y_{e+1} = h + c*k_e
OWN_LO, OWN_HI = 384, 896               # own out-cols

_CACHED_NC = None


def band_window(gi, j):
    """Nonzero band cols of S tile (group gi, y-chunk j), group-local."""
    glo, ghi = GROUPS[gi]
    gw = ghi - glo
    wl = max(0, 128 * j - glo - 256)
    wh = min(gw, 128 * j - glo + 128)
    return wl, wh


def tt_layout():
    """Packed layout of the static band profile: [(gi, j, wl, wh, off)]."""
    out = []
    off = 0
    for gi in range(len(GROUPS)):
        for j in JSETS[gi]:
            wl, wh = band_window(gi, j)
            out.append((gi, j, wl, wh, off))
            off += wh - wl
    return out, off


TT_SEGS, TT_W = tt_layout()


def active_js(gi, a, b):
    """j-chunks of group gi whose band overlaps [a,b) (group-local)."""
    out = []
    for j in JSETS[gi]:
        wl, wh = band_window(gi, j)
        if wl < wh and wl < b and wh > a:
            out.append(j)
    return out


def build_nc():
    nc = bacc.Bacc(
        "TRN2",
        target_bir_lowering=False,
        debug=False,
        enable_asserts=True,
        num_devices=NCORES,
    )

    xT_d = nc.dram_tensor("xT", [D_PAD, R], F16, kind="ExternalInput")
    wp_d = nc.dram_tensor("wp", [D_PAD, H], F16, kind="ExternalInput")
    w1a_d = nc.dram_tensor("w1a", [H, H], F16, kind="ExternalInput")
    w1b_d = nc.dram_tensor("w1b", [H, H], F16, kind="ExternalInput")
    w2_d = nc.dram_tensor("w2", [H, H], F16, kind="ExternalInput")
    bp_d = nc.dram_tensor("bp", [H, 1], F32, kind="ExternalInput")
    b1_d = nc.dram_tensor("b1", [H, 1], F32, kind="ExternalInput")
    b2c2_d = nc.dram_tensor("b2c", [H, 1], F32, kind="ExternalInput")
    ident_d = nc.dram_tensor("ident", [P, P], F16, kind="ExternalInput")
    ttp_d = nc.dram_tensor("ttp", [P, TT_W], F16, kind="ExternalInput")
    spki_d = nc.dram_tensor("spki", [P, X], F16, kind="ExternalInput")
    ami_d = nc.dram_tensor("ami", [P, X], F16, kind="ExternalInput")
    bmi_d = nc.dram_tensor("bmi", [P, X], F16, kind="ExternalInput")
    cmi_d = nc.dram_tensor("cmi", [P, X], F16, kind="ExternalInput")
    spkj_d = nc.dram_tensor("spkj", [P, NCH], F32, kind="ExternalInput")
    namj_d = nc.dram_tensor("namj", [P, NCH], F32, kind="ExternalInput")
    nbmj_d = nc.dram_tensor("nbmj", [P, NCH], F32, kind="ExternalInput")
    ncmj_d = nc.dram_tensor("ncmj", [P, NCH], F32, kind="ExternalInput")
    hidx_d = nc.dram_tensor("hidx", [1, 2], U32, kind="ExternalInput")

    out_d = nc.dram_tensor("hT_out", [H, R], F32, kind="ExternalOutput")

    with tile.TileContext(nc) as tc:
        with (
            tc.tile_pool(name="consts", bufs=1) as cs,
            tc.tile_pool(name="sbuild", bufs=1) as sbp,
            tc.tile_pool(name="work", bufs=2) as wk,
            tc.tile_pool(name="states", bufs=1) as st,
            tc.tile_pool(name="ps", bufs=1, space="PSUM") as ps,
            tc.tile_pool(name="dram", bufs=1, space="DRAM") as dram,
        ):
            # ---------- critical path to AG0: xT/wp DMA -> proj -> h0 -------
            wp_r = cs.tile([P, ND, H], F16, tag="wp_r")
            nc.sync.dma_start(wp_r[:], wp_d[:].rearrange("(n p) m -> p n m", p=P))
            bp_c = cs.tile([H, 1], F32, tag="bp")
            nc.sync.dma_start(bp_c[:], bp_d[:])
            xT_s = cs.tile([P, ND, R], F16, tag="xT")
            xv = xT_d[:].rearrange("(n p) m -> p n m", p=P)
            nc.sync.dma_start(xT_s[:, 0:5, :], xv[:, 0:5, :])
            nc.scalar.dma_start(xT_s[:, 5:10, :], xv[:, 5:10, :])
            nc.gpsimd.dma_start(xT_s[:, 10:ND, :], xv[:, 10:ND, :])
            hidx_sb = cs.tile([1, 2], U32, tag="hidx")
            nc.sync.dma_start(hidx_sb[:], hidx_d[:])
            regs_l = nc.alloc_registers("hl_reg")
            nc.regs_load(regs_l, hidx_sb[0:1, 0:1])
            hl_v = nc.snap(regs_l, donate=True)
            regs_r = nc.alloc_registers("hr_reg")
            nc.regs_load(regs_r, hidx_sb[0:1, 1:2])
            hr_v = nc.snap(regs_r, donate=True)

            h0_ps = ps.tile([P, R], F32, tag="k0")
            for d in range(ND):
                nc.tensor.matmul(h0_ps[:], wp_r[:, d, :], xT_s[:, d, :],
                                 start=(d == 0), stop=(d == ND - 1))
            hT = st.tile([P, R], F32, tag="hT")
            nc.scalar.activation(hT[:], h0_ps[:], AF.Identity, bias=bp_c[:],
                                 scale=1.0)
            h16 = st.tile([P, NY], F16, tag="h16")
            nc.vector.memset(h16[:, 0 : 4 * P], 0.0)
            nc.vector.memset(h16[:, 8 * P :], 0.0)
            nc.vector.tensor_copy(h16[:, 4 * P : 8 * P], hT[:])

            ag_in = dram.tile([P, R], F16, tag="ag_in")

            def do_ag(src, it):
                nc.sync.dma_start(ag_in[:], src)
                ag_out = dram.tile([NCORES * P, R], F16, tag=f"ago{it}",
                                   addr_space="Shared")
                nc.gpsimd.collective_compute(
                    "AllGather",
                    ALU.bypass,
                    replica_groups=[list(range(NCORES))],
                    ins=[ag_in[:].opt()],
                    outs=[ag_out[:].opt()],
                )
                return ag_out

            def fetch_halo(ag_out, dst):
                nc.scalar.dma_start(dst[:, 0 : 4 * P], ag_out[bass.ds(hl_v, P), :])
                nc.sync.dma_start(dst[:, 8 * P :], ag_out[bass.ds(hr_v, P), :])

            ag0 = do_ag(h16[:, 4 * P : 8 * P], 0)

            # ---------- remaining constants / tables ----------
            def load(dram_t, shape, name, dt=F16, eng=nc.scalar):
                t = cs.tile(shape, dt, tag=name)
                eng.dma_start(t[:], dram_t[:])
                return t

            w1a = load(w1a_d, [H, H], "w1a")
            w1b = load(w1b_d, [H, H], "w1b")
            w2 = load(w2_d, [H, H], "w2")
            b1_c = load(b1_d, [H, 1], "b1", F32)
            ident = load(ident_d, [P, P], "ident")
            ttp = load(ttp_d, [P, TT_W], "ttp", F16, nc.sync)
            spki = load(spki_d, [P, X], "spki", F16, nc.scalar)
            ami = load(ami_d, [P, X], "ami", F16, nc.scalar)
            bmi = load(bmi_d, [P, X], "bmi", F16, nc.scalar)
            cmi = load(cmi_d, [P, X], "cmi", F16, nc.scalar)
            spkj = load(spkj_d, [P, NCH], "spkj", F32, nc.scalar)
            namj = load(namj_d, [P, NCH], "namj", F32, nc.scalar)
            nbmj = load(nbmj_d, [P, NCH], "nbmj", F32, nc.scalar)
            ncmj = load(ncmj_d, [P, NCH], "ncmj", F32, nc.scalar)

            ones16 = cs.tile([P, 1], F16, tag="ones16")
            nc.vector.memset(ones16[:], 1.0)
            onesN = cs.tile([1, R], F16, tag="onesN")
            nc.vector.memset(onesN[:], 1.0)
            onesrow_f = cs.tile([1, P], F32, tag="onesrow_f")
            nc.vector.memset(onesrow_f[:], 1.0)
            onesrow = cs.tile([1, P], F32R, tag="onesrow")
            nc.vector.tensor_copy(onesrow[:], onesrow_f[:])
            b2c_d = cs.tile([H, 1], F32, tag="b2c")
            nc.sync.dma_start(b2c_d[:], b2c2_d[:])
            cb2h = cs.tile([H, 1], F32, tag="cb2h")
            nc.vector.tensor_scalar(cb2h[:], b2c_d[:], 0.5 * DT, None, ALU.mult)
            cb2f = cs.tile([H, 1], F32, tag="cb2f")
            nc.vector.tensor_scalar(cb2f[:], b2c_d[:], DT, None, ALU.mult)
            b2x6 = cs.tile([H, 1], F32, tag="b2x6")
            nc.vector.tensor_scalar(b2x6[:], b2c_d[:], 6.0, None, ALU.mult)

            with nc.allow_low_precision(reason="fp16 compute, f32 accumulate"):
                # ---------- banded S build (DVE + gpsimd) ----------
                s_raw = []     # per (gi, j): unnormalized fp16 tiles
                seg_off = {(gi, j): (wl, wh, off)
                           for gi, j, wl, wh, off in TT_SEGS}
                for gi, (glo, ghi) in enumerate(GROUPS):
                    gw = ghi - glo
                    row = {}
                    for j in JSETS[gi]:
                        t = sbp.tile([P, gw], F16, tag=f"sr{gi}_{j}")
                        wl, wh, off = seg_off[(gi, j)]
                        if wl > 0:
                            nc.gpsimd.memset(t[:, 0:wl], 0.0)
                        if wh < gw:
                            nc.gpsimd.memset(t[:, wh:gw], 0.0)
                        w = wh - wl
                        isl = slice(glo + wl, glo + wh)
                        jc = slice(j, j + 1)
                        da = wk.tile([P, w], F16, tag="da")
                        nc.scalar.activation(da[:], ami[:, isl], AF.Abs,
                                             bias=namj[:, jc], scale=1.0)
                        db = wk.tile([P, w], F16, tag="db")
                        nc.scalar.activation(db[:], bmi[:, isl], AF.Abs,
                                             bias=nbmj[:, jc], scale=1.0)
                        dc = wk.tile([P, w], F16, tag="dc")
                        nc.scalar.activation(dc[:], cmi[:, isl], AF.Abs,
                                             bias=ncmj[:, jc], scale=1.0)
                        e1 = wk.tile([P, w], F16, tag="e1")
                        nc.vector.tensor_tensor(e1[:], da[:], db[:], ALU.add)
                        e2 = wk.tile([P, w], F16, tag="e2")
                        nc.vector.tensor_tensor(e2[:], e1[:], dc[:], ALU.add)
                        uu = wk.tile([P, w], F16, tag="uu")
                        nc.vector.tensor_scalar(uu[:], e2[:], -1.0, A2,
                                                ALU.mult, ALU.add)
                        pm = wk.tile([P, w], F16, tag="pm")
                        nc.vector.tensor_scalar(pm[:], spki[:, isl],
                                                spkj[:, jc], A1, ALU.is_equal,
                                                ALU.mult)
                        qq = wk.tile([P, w], F16, tag="qq")
                        nc.vector.tensor_tensor(qq[:], uu[:], pm[:], ALU.max)
                        nc.vector.tensor_tensor(t[:, wl:wh],
                                                ttp[:, off : off + w], qq[:],
                                                ALU.mult)
                        row[j] = t
                    s_raw.append(row)

                # ---------- row sums -> fold 1/d into fp16 S ----------
                s_n = []
                for gi, (glo, ghi) in enumerate(GROUPS):
                    gw = ghi - glo
                    d_ps = ps.tile([P, 512], F32, tag="hn")
                    js = JSETS[gi]
                    for n, j in enumerate(js):
                        nc.tensor.matmul(d_ps[0:1, 0:gw], ones16[:],
                                         s_raw[gi][j][:],
                                         start=(n == 0), stop=(n == len(js) - 1))
                    dsum = wk.tile([1, gw], F32, tag="dsum")
                    nc.vector.tensor_scalar(dsum[:], d_ps[0:1, 0:gw], 1e-8,
                                            None, ALU.add)
                    rd = wk.tile([1, gw], F32R, tag="rd")
                    nc.vector.reciprocal(rd[:], dsum[:])
                    rdb_ps = ps.tile([P, 512], F32, tag="z1", bufs=2)
                    nc.tensor.matmul(rdb_ps[:, 0:gw], onesrow[:], rd[:])
                    row = {}
                    for j in JSETS[gi]:
                        t = cs.tile([P, gw], F16, tag=f"sn{gi}_{j}")
                        nc.vector.tensor_tensor(t[:], s_raw[gi][j][:],
                                                rdb_ps[:, 0:gw], ALU.mult)
                        row[j] = t
                    s_n.append(row)

                # ---------- h0 halo + initial row-form chunks ----------
                yr = st.tile([P, NCH, P], F16, tag="yr")

                def transpose_chunks(src16, chunks):
                    for c0 in range(0, len(chunks), 4):
                        grp = chunks[c0 : c0 + 4]
                        n = len(grp)
                        tp = ps.tile([P, 4, P], F16, tag="tr")
                        for k, ch in enumerate(grp):
                            nc.tensor.transpose(tp[:, k, :],
                                                src16[:, ch * P : (ch + 1) * P],
                                                ident[:])
                        nc.vector.tensor_copy(yr[:, grp[0] : grp[0] + n, :],
                                              tp[:, 0:n, :])

                transpose_chunks(h16, [4, 5, 6, 7])
                fetch_halo(ag0, h16)
                transpose_chunks(h16, [0, 1, 2, 3])
                transpose_chunks(h16, [8, 9, 10, 11])

                # ---------- RK4 loop ----------
                y16 = h16
                for step in range(N_STEPS):
                    for e in range(4):
                        lo, hi = EV_LO[e], EV_HI[e]
                        gidx = []
                        for gi, (glo, ghi) in enumerate(GROUPS):
                            a, b = max(lo, glo) - glo, min(hi, ghi) - glo
                            if a < b:
                                gidx.append((gi, a, b))
                        # software-pipelined per-group schedule:
                        # PE: S(g0), S(g1), z1(g0), S(g2), z1(g1), z2(g0),
                        #     z1(g2), z2(g1), z2(g2)
                        hns, hn16s, z1s, kps = {}, {}, {}, {}

                        def s_mm(gi, a, b):
                            js = active_js(gi, a, b)
                            hn = ps.tile([P, 512], F32, tag="hn")
                            for n, j in enumerate(js):
                                nc.tensor.matmul(hn[:, 0 : b - a], yr[:, j, :],
                                                 s_n[gi][j][:, a:b],
                                                 start=(n == 0),
                                                 stop=(n == len(js) - 1))
                            hns[gi] = hn
                            h16g = wk.tile([P, b - a], F16, tag="hn16")
                            nc.scalar.activation(h16g[:], hn[:, 0 : b - a],
                                                 AF.Copy, bias=0.0, scale=1.0)
                            hn16s[gi] = h16g

                        def z1_mm(gi, a, b):
                            glo = GROUPS[gi][0]
                            us = slice(glo + a + P, glo + b + P)
                            z1 = ps.tile([P, 512], F32, tag="z1", bufs=2)
                            nc.tensor.matmul(z1[:, 0 : b - a], w1a[:],
                                             y16[:, us], start=True, stop=False)
                            nc.tensor.matmul(z1[:, 0 : b - a], w1b[:],
                                             hn16s[gi][:], start=False,
                                             stop=True)
                            z1s[gi] = z1

                        y16n = None
                        if e < 3:
                            y16n = st.tile([P, NY], F16, tag="y16", bufs=2)
                            c = YCOEF[e]
                            cb2 = cb2f if e == 2 else cb2h

                        def z2_mm(gi, a, b):
                            th = wk.tile([P, b - a], F16, tag="th")
                            nc.scalar.activation(th[:], z1s[gi][:, 0 : b - a],
                                                 AF.Tanh, bias=b1_c[:],
                                                 scale=1.0)
                            kp = ps.tile([P, 512], F32, tag=f"k{gi}")
                            nc.tensor.matmul(kp[:, 0 : b - a], w2[:], th[:])
                            kps[gi] = (kp, a, b)
                            if e < 3:
                                # this group's y section + row-form chunks
                                glo = GROUPS[gi][0]
                                ys = slice(glo + a + P, glo + b + P)
                                yc = wk.tile([P, b - a], F16, tag="yc")
                                nc.scalar.activation(yc[:], kp[:, 0 : b - a],
                                                     AF.Identity,
                                                     bias=cb2[:], scale=c)
                                nc.vector.tensor_tensor(y16n[:, ys], yc[:],
                                                        h16[:, ys], ALU.add)
                                transpose_chunks(
                                    y16n,
                                    list(range((glo + a) // P + 1,
                                               (glo + b) // P + 1)))

                        stages = []
                        for n, g in enumerate(gidx):
                            stages.append((s_mm, g))
                            if n >= 1:
                                stages.append((z1_mm, gidx[n - 1]))
                            if n >= 2:
                                stages.append((z2_mm, gidx[n - 2]))
                        stages.append((z1_mm, gidx[-1]))
                        if len(gidx) >= 2:
                            stages.append((z2_mm, gidx[-2]))
                        stages.append((z2_mm, gidx[-1]))
                        for fn, g in stages:
                            fn(*g)

                        # RK4 accumulator (DVE; kp already includes b2)
                        own_pieces = []
                        for gi, (kp, a, b) in kps.items():
                            glo = GROUPS[gi][0]
                            oa, ob = max(OWN_LO, glo + a), min(OWN_HI, glo + b)
                            if oa < ob:
                                own_pieces.append(
                                    (kp[:, oa - glo - a : ob - glo - a],
                                     slice(oa - OWN_LO, ob - OWN_LO)))
                        if e == 0:
                            acc = st.tile([P, R], F32, tag="acc", bufs=2)
                            for kpsl, osl in own_pieces:
                                nc.vector.tensor_copy(acc[:, osl], kpsl)
                        elif e in (1, 2):
                            for kpsl, osl in own_pieces:
                                nc.vector.scalar_tensor_tensor(
                                    acc[:, osl], kpsl, 2.0, acc[:, osl],
                                    ALU.mult, ALU.add)
                        if e == 3:
                            last = step == N_STEPS - 1
                            hnew = st.tile([P, R], F32, tag=f"hT{step}")
                            for kpsl, osl in own_pieces:
                                a4 = wk.tile([P, osl.stop - osl.start], F32,
                                             tag="a4")
                                nc.vector.scalar_tensor_tensor(
                                    a4[:], kpsl, b2x6[:], acc[:, osl],
                                    ALU.add, ALU.add)
                                nc.vector.scalar_tensor_tensor(
                                    hnew[:, osl], a4[:], DT / 6.0, hT[:, osl],
                                    ALU.mult, ALU.add)
                            hT = hnew
                            if last:
                                nc.sync.dma_start(out_d[:], hT[:])
                                break
                            h16 = st.tile([P, NY], F16, tag=f"h16_{step}")
                            nc.vector.memset(h16[:, 0 : 4 * P], 0.0)
                            nc.vector.memset(h16[:, 8 * P :], 0.0)
                            nc.vector.tensor_copy(h16[:, 4 * P : 8 * P], hT[:])
                            ag = do_ag(h16[:, 4 * P : 8 * P], step + 1)
                            transpose_chunks(h16, [4, 5, 6, 7])
                            fetch_halo(ag, h16)
                            transpose_chunks(h16, [0, 1, 2, 3])
                            transpose_chunks(h16, [8, 9, 10, 11])
                            y16 = h16
                        else:
                            # y sections + transposes were produced
                            # per group inside z2_mm
                            y16 = y16n

    nc.compile()
    return nc


def get_nc():
    global _CACHED_NC
    if _CACHED_NC is None:
        _CACHED_NC = build_nc()
    return _CACHED_NC


def prep_inputs(features, speaker_ids, modality_masks, Wp, bp, W1, b1, W2, b2):
    features = np.asarray(features, dtype=np.float32)
    spk = np.asarray(speaker_ids).astype(np.float32)
    mm = np.asarray(modality_masks, dtype=np.float32)
    Wp = np.asarray(Wp, dtype=np.float32)
    bp = np.asarray(bp, dtype=np.float32)
    W1 = np.asarray(W1, dtype=np.float32)
    b1 = np.asarray(b1, dtype=np.float32)
    W2 = np.asarray(W2, dtype=np.float32)
    b2 = np.asarray(b2, dtype=np.float32)

    wp_pad = np.zeros((D_PAD, H), dtype=np.float16)
    wp_pad[:D_IN] = Wp.astype(np.float16)
    msc = (mm * (A2 / 3.0)).astype(np.float32)
    ident = np.eye(P, dtype=np.float16)

    def repi(v, dt=np.float16):
        return np.ascontiguousarray(
            np.broadcast_to(v, (P, v.shape[0])), dtype=dt)

    in_maps = []
    for c in range(NCORES):
        own0 = c * R
        gx = own0 - 384 + np.arange(X)
        gxc = np.clip(gx, 0, B - 1)
        jv = own0 - 512 + np.arange(NY)
        jvalid = (jv >= 0) & (jv < B)
        jvc = np.clip(jv, 0, B - 1)
        # static band profile, packed over (gi, j) segments
        ttp = np.zeros((P, TT_W), dtype=np.float16)
        for gi, j, wl, wh, off in TT_SEGS:
            glo = GROUPS[gi][0]
            iv = gx[glo + wl : glo + wh][None, :]          # [1, w]
            jvv = jv[j * P : (j + 1) * P][:, None]         # [P, 1]
            val = np.exp(-BETA * np.abs(iv - jvv))
            val = np.where(iv == jvv, 1.0 / A1, val)       # diag fix
            jok = jvalid[j * P : (j + 1) * P][:, None]
            ttp[:, off : off + (wh - wl)] = np.where(jok, val, 0.0)
        xT = np.zeros((D_PAD, R), dtype=np.float16)
        xT[:D_IN] = features[own0 : own0 + R].T.astype(np.float16)
        hl_idx = P * (c - 1 if c > 0 else c)
        hr_idx = P * (c + 1 if c < NCORES - 1 else c)
        in_maps.append({
            "xT": xT,
            "wp": wp_pad,
            "w1a": W1[:H].astype(np.float16),
            "w1b": W1[H:].astype(np.float16),
            "w2": W2.astype(np.float16),
            "bp": bp.reshape(H, 1).astype(np.float32),
            "b1": b1.reshape(H, 1).astype(np.float32),
            "b2c": b2.reshape(H, 1).astype(np.float32),
            "ident": ident,
            "ttp": ttp,
            "spki": repi(spk[gxc].astype(np.float16)),
            "ami": repi(msc[gxc, 0].astype(np.float16)),
            "bmi": repi(msc[gxc, 1].astype(np.float16)),
            "cmi": repi(msc[gxc, 2].astype(np.float16)),
            "spkj": np.ascontiguousarray(
                spk[jvc].reshape(NCH, P).T, dtype=np.float32),
            "namj": np.ascontiguousarray(
                -msc[jvc, 0].reshape(NCH, P).T, dtype=np.float32),
            "nbmj": np.ascontiguousarray(
                -msc[jvc, 1].reshape(NCH, P).T, dtype=np.float32),
            "ncmj": np.ascontiguousarray(
                -msc[jvc, 2].reshape(NCH, P).T, dtype=np.float32),
            "hidx": np.array([[hl_idx, hr_idx]], dtype=np.uint32),
        })
    return in_maps


def kernel(features, speaker_ids, modality_masks, Wp, bp, W1, b1, W2, b2,
           _runner=None):
    in_maps = prep_inputs(features, speaker_ids, modality_masks,
                          Wp, bp, W1, b1, W2, b2)
    nc = get_nc()
    if _runner is not None:
        results = _runner(nc, in_maps)
    else:
        results = run_bass_kernel_spmd(nc, in_maps, list(range(NCORES))).results
    out = np.concatenate([results[c]["hT_out"].T for c in range(NCORES)], axis=0)
    return np.ascontiguousarray(out, dtype=np.float32)


# revision 14
# speedup vs baseline: 1.3422x; 1.3422x over previous
"""Trainium2 Bass kernel for nn_DGODE (graph ODE over utterance nodes).

Self-contained: hardcodes all shapes. Strategy (v2):
- Row-shard B=4096 nodes over 8 cores (512 rows each). The adjacency is
  effectively banded (exp(-0.1|i-j|) decay): band half-width 128.
- One AllGather per RK4 step boundary (4 total incl. the h0 exchange)
  instead of one per ODE eval: each core redundantly computes k1..k4 on a
  shrinking halo (+-384, +-256, +-128, 0), so no mid-step communication.
- All matmuls in fp16 (f32 PSUM accumulate); dense PE schedule to hold
  the high pstate; b2 bias applied as a rank-1 matmul on the PE.
- S built banded in T-form per (out-group, j-chunk) tile on DVE/gpsimd;
  the static exp(-beta|i-j|) band profile (index-only) is a host table,
  with the diagonal correction folded in (1/A1 trick); row-normalization
  folded into the final fp16 cast.
"""

import sys

if "/opt/trn_rl_repo" not in sys.path:
    sys.path.insert(0, "/opt/trn_rl_repo")

import numpy as np

import concourse.bacc as bacc
import concourse.bass as bass
import concourse.mybir as mybir
import concourse.tile as tile
from concourse.bass_utils import run_bass_kernel_spmd

F32 = mybir.dt.float32
F32R = mybir.dt.float32r
F16 = mybir.dt.float16
U32 = mybir.dt.uint32
AF = mybir.ActivationFunctionType
ALU = mybir.AluOpType

NCORES = 8
B = 4096
D_IN = 1856
D_PAD = 1920           # 15 * 128
ND = D_PAD // 128
H = 128
R = B // NCORES        # 512 rows per core
P = 128
N_STEPS = 4
DT = 1.0 / N_STEPS
A1, A2, BETA = 0.8, 0.5, 0.1

X = 1280               # out-col space: own +-384 (x=0 -> global own_start-384)
NY = 1536              # y space: own +-512, 12 chunks
NCH = NY // P          # 12 y-chunks
GROUPS = [(0, 512), (512, 1024), (1024, 1280)]
JSETS = [list(range(0, 6)), list(range(4, 10)), list(range(8, 12))]

EV_LO = [0, 128, 256, 384]
EV_HI = [1280, 1152, 1024, 896]
YCOEF = [0.5 * DT, 0.5 * DT, DT]        # y_{e+1} = h + c*k_e
OWN_LO, OWN_HI = 384, 896               # own out-cols

_CACHED_NC = None


def band_window(gi, j):
    """Nonzero band cols of S tile (group gi, y-chunk j), group-local."""
    glo, ghi = GROUPS[gi]
    gw = ghi - glo
    wl = max(0, 128 * j - glo - 256)
    wh = min(gw, 128 * j - glo + 128)
    return wl, wh


def tt_layout():
    """Packed layout of the static band profile: [(gi, j, wl, wh, off)]."""
    out = []
    off = 0
    for gi in range(len(GROUPS)):
        for j in JSETS[gi]:
            wl, wh = band_window(gi, j)
            out.append((gi, j, wl, wh, off))
            off += wh - wl
    return out, off


TT_SEGS, TT_W = tt_layout()


def active_js(gi, a, b):
    """j-chunks of group gi whose band overlaps [a,b) (group-local)."""
    out = []
    for j in JSETS[gi]:
        wl, wh = band_window(gi, j)
        if wl < wh and wl < b and wh > a:
            out.append(j)
    return out


def build_nc():
    nc = bacc.Bacc(
        "TRN2",
        target_bir_lowering=False,
        debug=False,
        enable_asserts=True,
        num_devices=NCORES,
    )

    xT_d = nc.dram_tensor("xT", [D_PAD, R], F16, kind="ExternalInput")
    wp_d = nc.dram_tensor("wp", [D_PAD, H], F16, kind="ExternalInput")
    w1a_d = nc.dram_tensor("w1a", [H, H], F16, kind="ExternalInput")
    w1b_d = nc.dram_tensor("w1b", [H, H], F16, kind="ExternalInput")
    w2_d = nc.dram_tensor("w2", [H, H], F16, kind="ExternalInput")
    bp_d = nc.dram_tensor("bp", [H, 1], F32, kind="ExternalInput")
    b1_d = nc.dram_tensor("b1", [H, 1], F32, kind="ExternalInput")
    b2c2_d = nc.dram_tensor("b2c", [H, 1], F32, kind="ExternalInput")
    b2r_d = nc.dram_tensor("b2r", [1, H], F16, kind="ExternalInput")
    ident_d = nc.dram_tensor("ident", [P, P], F16, kind="ExternalInput")
    ttp_d = nc.dram_tensor("ttp", [P, TT_W], F16, kind="ExternalInput")
    spki_d = nc.dram_tensor("spki", [P, X], F16, kind="ExternalInput")
    ami_d = nc.dram_tensor("ami", [P, X], F16, kind="ExternalInput")
    bmi_d = nc.dram_tensor("bmi", [P, X], F16, kind="ExternalInput")
    cmi_d = nc.dram_tensor("cmi", [P, X], F16, kind="ExternalInput")
    spkj_d = nc.dram_tensor("spkj", [P, NCH], F32, kind="ExternalInput")
    namj_d = nc.dram_tensor("namj", [P, NCH], F32, kind="ExternalInput")
    nbmj_d = nc.dram_tensor("nbmj", [P, NCH], F32, kind="ExternalInput")
    ncmj_d = nc.dram_tensor("ncmj", [P, NCH], F32, kind="ExternalInput")
    hidx_d = nc.dram_tensor("hidx", [1, 2], U32, kind="ExternalInput")

    out_d = nc.dram_tensor("hT_out", [H, R], F32, kind="ExternalOutput")

    with tile.TileContext(nc) as tc:
        with (
            tc.tile_pool(name="consts", bufs=1) as cs,
            tc.tile_pool(name="sbuild", bufs=1) as sbp,
            tc.tile_pool(name="work", bufs=2) as wk,
            tc.tile_pool(name="states", bufs=1) as st,
            tc.tile_pool(name="ps", bufs=1, space="PSUM") as ps,
            tc.tile_pool(name="dram", bufs=1, space="DRAM") as dram,
        ):
            # ---------- critical path to AG0: xT/wp DMA -> proj -> h0 -------
            wp_r = cs.tile([P, ND, H], F16, tag="wp_r")
            nc.sync.dma_start(wp_r[:], wp_d[:].rearrange("(n p) m -> p n m", p=P))
            bp_c = cs.tile([H, 1], F32, tag="bp")
            nc.sync.dma_start(bp_c[:], bp_d[:])
            xT_s = cs.tile([P, ND, R], F16, tag="xT")
            xv = xT_d[:].rearrange("(n p) m -> p n m", p=P)
            nc.sync.dma_start(xT_s[:, 0:5, :], xv[:, 0:5, :])
            nc.scalar.dma_start(xT_s[:, 5:10, :], xv[:, 5:10, :])
            nc.gpsimd.dma_start(xT_s[:, 10:ND, :], xv[:, 10:ND, :])
            hidx_sb = cs.tile([1, 2], U32, tag="hidx")
            nc.sync.dma_start(hidx_sb[:], hidx_d[:])
            regs_l = nc.alloc_registers("hl_reg")
            nc.regs_load(regs_l, hidx_sb[0:1, 0:1])
            hl_v = nc.snap(regs_l, donate=True)
            regs_r = nc.alloc_registers("hr_reg")
            nc.regs_load(regs_r, hidx_sb[0:1, 1:2])
            hr_v = nc.snap(regs_r, donate=True)

            h0_ps = ps.tile([P, R], F32, tag="k0")
            for d in range(ND):
                nc.tensor.matmul(h0_ps[:], wp_r[:, d, :], xT_s[:, d, :],
                                 start=(d == 0), stop=(d == ND - 1))
            hT = st.tile([P, R], F32, tag="hT")
            nc.scalar.activation(hT[:], h0_ps[:], AF.Identity, bias=bp_c[:],
                                 scale=1.0)
            h16 = st.tile([P, NY], F16, tag="h16")
            nc.vector.memset(h16[:, 0 : 4 * P], 0.0)
            nc.vector.memset(h16[:, 8 * P :], 0.0)
            nc.vector.tensor_copy(h16[:, 4 * P : 8 * P], hT[:])

            ag_in = dram.tile([P, R], F16, tag="ag_in")

            def do_ag(src, it):
                nc.sync.dma_start(ag_in[:], src)
                ag_out = dram.tile([NCORES * P, R], F16, tag=f"ago{it}",
                                   addr_space="Shared")
                nc.gpsimd.collective_compute(
                    "AllGather",
                    ALU.bypass,
                    replica_groups=[list(range(NCORES))],
                    ins=[ag_in[:].opt()],
                    outs=[ag_out[:].opt()],
                )
                return ag_out

            def fetch_halo(ag_out, dst):
                nc.scalar.dma_start(dst[:, 0 : 4 * P], ag_out[bass.ds(hl_v, P), :])
                nc.sync.dma_start(dst[:, 8 * P :], ag_out[bass.ds(hr_v, P), :])

            ag0 = do_ag(h16[:, 4 * P : 8 * P], 0)

            # ---------- remaining constants / tables ----------
            def load(dram_t, shape, name, dt=F16, eng=nc.scalar):
                t = cs.tile(shape, dt, tag=name)
                eng.dma_start(t[:], dram_t[:])
                return t

            w1a = load(w1a_d, [H, H], "w1a")
            b2r = load(b2r_d, [1, H], "b2r")
            w1b = load(w1b_d, [H, H], "w1b")
            w2 = load(w2_d, [H, H], "w2")
            b1_c = load(b1_d, [H, 1], "b1", F32)
            ident = load(ident_d, [P, P], "ident")
            ttp = load(ttp_d, [P, TT_W], "ttp", F16, nc.sync)
            spki = load(spki_d, [P, X], "spki", F16, nc.scalar)
            ami = load(ami_d, [P, X], "ami", F16, nc.scalar)
            bmi = load(bmi_d, [P, X], "bmi", F16, nc.scalar)
            cmi = load(cmi_d, [P, X], "cmi", F16, nc.scalar)
            spkj = load(spkj_d, [P, NCH], "spkj", F32, nc.scalar)
            namj = load(namj_d, [P, NCH], "namj", F32, nc.scalar)
            nbmj = load(nbmj_d, [P, NCH], "nbmj", F32, nc.scalar)
            ncmj = load(ncmj_d, [P, NCH], "ncmj", F32, nc.scalar)

            ones16 = cs.tile([P, 1], F16, tag="ones16")
            nc.vector.memset(ones16[:], 1.0)
            onesN = cs.tile([1, R], F16, tag="onesN")
            nc.vector.memset(onesN[:], 1.0)
            onesrow_f = cs.tile([1, P], F32, tag="onesrow_f")
            nc.vector.memset(onesrow_f[:], 1.0)
            onesrow = cs.tile([1, P], F32R, tag="onesrow")
            nc.vector.tensor_copy(onesrow[:], onesrow_f[:])
            b2c_d = cs.tile([H, 1], F32, tag="b2c")
            nc.sync.dma_start(b2c_d[:], b2c2_d[:])
            cb2h = cs.tile([H, 1], F32, tag="cb2h")
            nc.vector.tensor_scalar(cb2h[:], b2c_d[:], 0.5 * DT, None, ALU.mult)
            cb2f = cs.tile([H, 1], F32, tag="cb2f")
            nc.vector.tensor_scalar(cb2f[:], b2c_d[:], DT, None, ALU.mult)
            b2x6 = cs.tile([H, 1], F32, tag="b2x6")
            nc.vector.tensor_scalar(b2x6[:], b2c_d[:], 6.0, None, ALU.mult)

            with nc.allow_low_precision(reason="fp16 compute, f32 accumulate"):
                # ---------- banded S build (DVE + gpsimd) ----------
                s_raw = []     # per (gi, j): unnormalized fp16 tiles
                seg_off = {(gi, j): (wl, wh, off)
                           for gi, j, wl, wh, off in TT_SEGS}
                for gi, (glo, ghi) in enumerate(GROUPS):
                    gw = ghi - glo
                    row = {}
                    for j in JSETS[gi]:
                        t = sbp.tile([P, gw], F16, tag=f"sr{gi}_{j}")
                        wl, wh, off = seg_off[(gi, j)]
                        if wl > 0:
                            nc.gpsimd.memset(t[:, 0:wl], 0.0)
                        if wh < gw:
                            nc.gpsimd.memset(t[:, wh:gw], 0.0)
                        w = wh - wl
                        isl = slice(glo + wl, glo + wh)
                        jc = slice(j, j + 1)
                        da = wk.tile([P, w], F16, tag="da")
                        nc.scalar.activation(da[:], ami[:, isl], AF.Abs,
                                             bias=namj[:, jc], scale=1.0)
                        db = wk.tile([P, w], F16, tag="db")
                        nc.scalar.activation(db[:], bmi[:, isl], AF.Abs,
                                             bias=nbmj[:, jc], scale=1.0)
                        dc = wk.tile([P, w], F16, tag="dc")
                        nc.scalar.activation(dc[:], cmi[:, isl], AF.Abs,
                                             bias=ncmj[:, jc], scale=1.0)
                        e1 = wk.tile([P, w], F16, tag="e1")
                        nc.vector.tensor_tensor(e1[:], da[:], db[:], ALU.add)
                        e2 = wk.tile([P, w], F16, tag="e2")
                        nc.vector.tensor_tensor(e2[:], e1[:], dc[:], ALU.add)
                        uu = wk.tile([P, w], F16, tag="uu")
                        nc.vector.tensor_scalar(uu[:], e2[:], -1.0, A2,
                                                ALU.mult, ALU.add)
                        pm = wk.tile([P, w], F16, tag="pm")
                        nc.vector.tensor_scalar(pm[:], spki[:, isl],
                                                spkj[:, jc], A1, ALU.is_equal,
                                                ALU.mult)
                        qq = wk.tile([P, w], F16, tag="qq")
                        nc.vector.tensor_tensor(qq[:], uu[:], pm[:], ALU.max)
                        nc.vector.tensor_tensor(t[:, wl:wh],
                                                ttp[:, off : off + w], qq[:],
                                                ALU.mult)
                        row[j] = t
                    s_raw.append(row)

                # ---------- row sums -> fold 1/d into fp16 S ----------
                s_n = []
                for gi, (glo, ghi) in enumerate(GROUPS):
                    gw = ghi - glo
                    d_ps = ps.tile([P, 512], F32, tag="hn")
                    js = JSETS[gi]
                    for n, j in enumerate(js):
                        nc.tensor.matmul(d_ps[0:1, 0:gw], ones16[:],
                                         s_raw[gi][j][:],
                                         start=(n == 0), stop=(n == len(js) - 1))
                    dsum = wk.tile([1, gw], F32, tag="dsum")
                    nc.vector.tensor_scalar(dsum[:], d_ps[0:1, 0:gw], 1e-8,
                                            None, ALU.add)
                    rd = wk.tile([1, gw], F32R, tag="rd")
                    nc.vector.reciprocal(rd[:], dsum[:])
                    rdb_ps = ps.tile([P, 512], F32, tag="z1", bufs=2)
                    nc.tensor.matmul(rdb_ps[:, 0:gw], onesrow[:], rd[:])
                    row = {}
                    for j in JSETS[gi]:
                        t = cs.tile([P, gw], F16, tag=f"sn{gi}_{j}")
                        nc.vector.tensor_tensor(t[:], s_raw[gi][j][:],
                                                rdb_ps[:, 0:gw], ALU.mult)
                        row[j] = t
                    s_n.append(row)

                # ---------- h0 halo + initial row-form chunks ----------
                yr = st.tile([P, NCH, P], F16, tag="yr")

                def transpose_chunks(src16, chunks):
                    for c0 in range(0, len(chunks), 4):
                        grp = chunks[c0 : c0 + 4]
                        n = len(grp)
                        tp = ps.tile([P, 4, P], F16, tag="tr")
                        for k, ch in enumerate(grp):
                            nc.tensor.transpose(tp[:, k, :],
                                                src16[:, ch * P : (ch + 1) * P],
                                                ident[:])
                        nc.vector.tensor_copy(yr[:, grp[0] : grp[0] + n, :],
                                              tp[:, 0:n, :])

                transpose_chunks(h16, [4, 5, 6, 7])
                fetch_halo(ag0, h16)
                transpose_chunks(h16, [0, 1, 2, 3])
                transpose_chunks(h16, [8, 9, 10, 11])

                # ---------- RK4 loop ----------
                y16 = h16
                for step in range(N_STEPS):
                    for e in range(4):
                        lo, hi = EV_LO[e], EV_HI[e]
                        gidx = []
                        for gi, (glo, ghi) in enumerate(GROUPS):
                            a, b = max(lo, glo) - glo, min(hi, ghi) - glo
                            if a < b:
                                gidx.append((gi, a, b))
                        # software-pipelined per-group schedule:
                        # PE: S(g0), S(g1), z1(g0), S(g2), z1(g1), z2(g0),
                        #     z1(g2), z2(g1), z2(g2)
                        hns, hn16s, z1s, kps = {}, {}, {}, {}

                        def s_mm(gi, a, b):
                            js = active_js(gi, a, b)
                            hn = ps.tile([P, 512], F32, tag="hn")
                            for n, j in enumerate(js):
                                nc.tensor.matmul(hn[:, 0 : b - a], yr[:, j, :],
                                                 s_n[gi][j][:, a:b],
                                                 start=(n == 0),
                                                 stop=(n == len(js) - 1))
                            hns[gi] = hn
                            h16g = wk.tile([P, b - a], F16, tag="hn16")
                            nc.scalar.activation(h16g[:], hn[:, 0 : b - a],
                                                 AF.Copy, bias=0.0, scale=1.0)
                            hn16s[gi] = h16g

                        def z1_mm(gi, a, b):
                            glo = GROUPS[gi][0]
                            us = slice(glo + a + P, glo + b + P)
                            z1 = ps.tile([P, 512], F32, tag="z1", bufs=2)
                            nc.tensor.matmul(z1[:, 0 : b - a], w1a[:],
                                             y16[:, us], start=True, stop=False)
                            nc.tensor.matmul(z1[:, 0 : b - a], w1b[:],
                                             hn16s[gi][:], start=False,
                                             stop=True)
                            z1s[gi] = z1

                        # 1) all S matmuls (dense PE run)
                        hns = {}
                        for gi, a, b in gidx:
                            js = active_js(gi, a, b)
                            hn = ps.tile([P, 512], F32, tag="hn")
                            for n, j in enumerate(js):
                                nc.tensor.matmul(hn[:, 0 : b - a], yr[:, j, :],
                                                 s_n[gi][j][:, a:b],
                                                 start=(n == 0),
                                                 stop=(n == len(js) - 1))
                            hns[gi] = hn
                        hn16s = {}
                        for gi, a, b in gidx:
                            h16g = wk.tile([P, b - a], F16, tag="hn16")
                            nc.scalar.activation(h16g[:], hns[gi][:, 0 : b - a],
                                                 AF.Copy, bias=0.0, scale=1.0)
                            hn16s[gi] = h16g
                        z1s = {}
                        for gi, a, b in gidx:
                            glo = GROUPS[gi][0]
                            us = slice(glo + a + P, glo + b + P)
                            z1 = ps.tile([P, 512], F32, tag="z1", bufs=2)
                            nc.tensor.matmul(z1[:, 0 : b - a], w1a[:],
                                             y16[:, us], start=True, stop=False)
                            nc.tensor.matmul(z1[:, 0 : b - a], w1b[:],
                                             hn16s[gi][:], start=False,
                                             stop=True)
                            z1s[gi] = z1
                        kps = {}
                        for gi, a, b in gidx:
                            th = wk.tile([P, b - a], F16, tag="th")
                            nc.scalar.activation(th[:], z1s[gi][:, 0 : b - a],
                                                 AF.Tanh, bias=b1_c[:],
                                                 scale=1.0)
                            kp = ps.tile([P, 512], F32, tag=f"k{gi}")
                            nc.tensor.matmul(kp[:, 0 : b - a], w2[:], th[:],
                                             start=True, stop=False)
                            nc.tensor.matmul(kp[:, 0 : b - a], b2r[:],
                                             onesN[:, 0 : b - a], start=False,
                                             stop=True)
                            kps[gi] = (kp, a, b)

                        # RK4 accumulator (DVE; kp already includes b2)
                        own_pieces = []
                        for gi, (kp, a, b) in kps.items():
                            glo = GROUPS[gi][0]
                            oa, ob = max(OWN_LO, glo + a), min(OWN_HI, glo + b)
                            if oa < ob:
                                own_pieces.append(
                                    (kp[:, oa - glo - a : ob - glo - a],
                                     slice(oa - OWN_LO, ob - OWN_LO)))
                        if e == 0:
                            acc = st.tile([P, R], F32, tag="acc", bufs=2)
                            for kpsl, osl in own_pieces:
                                nc.vector.tensor_copy(acc[:, osl], kpsl)
                        elif e in (1, 2):
                            for kpsl, osl in own_pieces:
                                nc.vector.scalar_tensor_tensor(
                                    acc[:, osl], kpsl, 2.0, acc[:, osl],
                                    ALU.mult, ALU.add)
                        if e == 3:
                            last = step == N_STEPS - 1
                            hnew = st.tile([P, R], F32, tag=f"hT{step}")
                            for kpsl, osl in own_pieces:
                                a4 = wk.tile([P, osl.stop - osl.start], F32,
                                             tag="a4")
                                nc.vector.tensor_tensor(a4[:], kpsl,
                                                        acc[:, osl], ALU.add)
                                nc.vector.scalar_tensor_tensor(
                                    hnew[:, osl], a4[:], DT / 6.0, hT[:, osl],
                                    ALU.mult, ALU.add)
                            hT = hnew
                            if last:
                                nc.sync.dma_start(out_d[:], hT[:])
                                break
                            h16 = st.tile([P, NY], F16, tag=f"h16_{step}")
                            nc.vector.memset(h16[:, 0 : 4 * P], 0.0)
                            nc.vector.memset(h16[:, 8 * P :], 0.0)
                            nc.vector.tensor_copy(h16[:, 4 * P : 8 * P], hT[:])
                            ag = do_ag(h16[:, 4 * P : 8 * P], step + 1)
                            transpose_chunks(h16, [4, 5, 6, 7])
                            fetch_halo(ag, h16)
                            transpose_chunks(h16, [0, 1, 2, 3])
                            transpose_chunks(h16, [8, 9, 10, 11])
                            y16 = h16
                        else:
                            c = YCOEF[e]
                            nlo, nhi = EV_LO[e + 1], EV_HI[e + 1] + 2 * P
                            y16 = st.tile([P, NY], F16, tag="y16", bufs=2)
                            for gi, (kp, a, b) in kps.items():
                                glo = GROUPS[gi][0]
                                ys = slice(glo + a + P, glo + b + P)
                                nc.vector.scalar_tensor_tensor(
                                    y16[:, ys], kp[:, 0 : b - a], c,
                                    h16[:, ys], ALU.mult, ALU.add)
                            transpose_chunks(y16, list(range(nlo // P,
                                                             nhi // P)))

    nc.compile()
    return nc


def get_nc():
    global _CACHED_NC
    if _CACHED_NC is None:
        _CACHED_NC = build_nc()
    return _CACHED_NC


def prep_inputs(features, speaker_ids, modality_masks, Wp, bp, W1, b1, W2, b2):
    features = np.asarray(features, dtype=np.float32)
    spk = np.asarray(speaker_ids).astype(np.float32)
    mm = np.asarray(modality_masks, dtype=np.float32)
    Wp = np.asarray(Wp, dtype=np.float32)
    bp = np.asarray(bp, dtype=np.float32)
    W1 = np.asarray(W1, dtype=np.float32)
    b1 = np.asarray(b1, dtype=np.float32)
    W2 = np.asarray(W2, dtype=np.float32)
    b2 = np.asarray(b2, dtype=np.float32)

    wp_pad = np.zeros((D_PAD, H), dtype=np.float16)
    wp_pad[:D_IN] = Wp.astype(np.float16)
    msc = (mm * (A2 / 3.0)).astype(np.float32)
    ident = np.eye(P, dtype=np.float16)

    def repi(v, dt=np.float16):
        return np.ascontiguousarray(
            np.broadcast_to(v, (P, v.shape[0])), dtype=dt)

    in_maps = []
    for c in range(NCORES):
        own0 = c * R
        gx = own0 - 384 + np.arange(X)
        gxc = np.clip(gx, 0, B - 1)
        jv = own0 - 512 + np.arange(NY)
        jvalid = (jv >= 0) & (jv < B)
        jvc = np.clip(jv, 0, B - 1)
        # static band profile, packed over (gi, j) segments
        ttp = np.zeros((P, TT_W), dtype=np.float16)
        for gi, j, wl, wh, off in TT_SEGS:
            glo = GROUPS[gi][0]
            iv = gx[glo + wl : glo + wh][None, :]          # [1, w]
            jvv = jv[j * P : (j + 1) * P][:, None]         # [P, 1]
            val = np.exp(-BETA * np.abs(iv - jvv))
            val = np.where(iv == jvv, 1.0 / A1, val)       # diag fix
            jok = jvalid[j * P : (j + 1) * P][:, None]
            ttp[:, off : off + (wh - wl)] = np.where(jok, val, 0.0)
        xT = np.zeros((D_PAD, R), dtype=np.float16)
        xT[:D_IN] = features[own0 : own0 + R].T.astype(np.float16)
        hl_idx = P * (c - 1 if c > 0 else c)
        hr_idx = P * (c + 1 if c < NCORES - 1 else c)
        in_maps.append({
            "xT": xT,
            "wp": wp_pad,
            "w1a": W1[:H].astype(np.float16),
            "w1b": W1[H:].astype(np.float16),
            "w2": W2.astype(np.float16),
            "bp": bp.reshape(H, 1).astype(np.float32),
            "b1": b1.reshape(H, 1).astype(np.float32),
            "b2c": b2.reshape(H, 1).astype(np.float32),
            "b2r": b2.reshape(1, H).astype(np.float16),
            "ident": ident,
            "ttp": ttp,
            "spki": repi(spk[gxc].astype(np.float16)),
            "ami": repi(msc[gxc, 0].astype(np.float16)),
            "bmi": repi(msc[gxc, 1].astype(np.float16)),
            "cmi": repi(msc[gxc, 2].astype(np.float16)),
            "spkj": np.ascontiguousarray(
                spk[jvc].reshape(NCH, P).T, dtype=np.float32),
            "namj": np.ascontiguousarray(
                -msc[jvc, 0].reshape(NCH, P).T, dtype=np.float32),
            "nbmj": np.ascontiguousarray(
                -msc[jvc, 1].reshape(NCH, P).T, dtype=np.float32),
            "ncmj": np.ascontiguousarray(
                -msc[jvc, 2].reshape(NCH, P).T, dtype=np.float32),
            "hidx": np.array([[hl_idx, hr_idx]], dtype=np.uint32),
        })
    return in_maps


def kernel(features, speaker_ids, modality_masks, Wp, bp, W1, b1, W2, b2,
           _runner=None):
    in_maps = prep_inputs(features, speaker_ids, modality_masks,
                          Wp, bp, W1, b1, W2, b2)
    nc = get_nc()
    if _runner is not None:
        results = _runner(nc, in_maps)
    else:
        results = run_bass_kernel_spmd(nc, in_maps, list(range(NCORES))).results
    out = np.concatenate([results[c]["hT_out"].T for c in range(NCORES)], axis=0)
    return np.ascontiguousarray(out, dtype=np.float32)


# revision 15
# speedup vs baseline: 1.3789x; 1.0273x over previous
"""Trainium2 Bass kernel for nn_DGODE (graph ODE over utterance nodes).

Self-contained: hardcodes all shapes. Strategy (v2):
- Row-shard B=4096 nodes over 8 cores (512 rows each). The adjacency is
  effectively banded (exp(-0.1|i-j|) decay): band half-width 128.
- One AllGather per RK4 step boundary (4 total incl. the h0 exchange)
  instead of one per ODE eval: each core redundantly computes k1..k4 on a
  shrinking halo (+-384, +-256, +-128, 0), so no mid-step communication.
- All matmuls in fp16 (f32 PSUM accumulate); dense PE schedule to hold
  the high pstate; b2 bias applied as a rank-1 matmul on the PE.
- S built banded in T-form per (out-group, j-chunk) tile on DVE/gpsimd;
  the static exp(-beta|i-j|) band profile (index-only) is a host table,
  with the diagonal correction folded in (1/A1 trick); row-normalization
  folded into the final fp16 cast.
"""

import sys

if "/opt/trn_rl_repo" not in sys.path:
    sys.path.insert(0, "/opt/trn_rl_repo")

import numpy as np

import concourse.bacc as bacc
import concourse.bass as bass
import concourse.mybir as mybir
import concourse.tile as tile
from concourse.bass_utils import run_bass_kernel_spmd

F32 = mybir.dt.float32
F32R = mybir.dt.float32r
F16 = mybir.dt.float16
U32 = mybir.dt.uint32
AF = mybir.ActivationFunctionType
ALU = mybir.AluOpType

NCORES = 8
B = 4096
D_IN = 1856
D_PAD = 1920           # 15 * 128
ND = D_PAD // 128
H = 128
R = B // NCORES        # 512 rows per core
P = 128
N_STEPS = 4
DT = 1.0 / N_STEPS
A1, A2, BETA = 0.8, 0.5, 0.1

X = 1280               # out-col space: own +-384 (x=0 -> global own_start-384)
NY = 1536              # y space: own +-512, 12 chunks
NCH = NY // P          # 12 y-chunks
GROUPS = [(0, 512), (512, 1024), (1024, 1280)]
JSETS = [list(range(0, 6)), list(range(4, 10)), list(range(8, 12))]

EV_LO = [0, 128, 256, 384]
EV_HI = [1280, 1152, 1024, 896]
YCOEF = [0.5 * DT, 0.5 * DT, DT]        # y_{e+1} = h + c*k_e
OWN_LO, OWN_HI = 384, 896               # own out-cols

_CACHED_NC = None


def band_window(gi, j):
    """Nonzero band cols of S tile (group gi, y-chunk j), group-local."""
    glo, ghi = GROUPS[gi]
    gw = ghi - glo
    wl = max(0, 128 * j - glo - 256)
    wh = min(gw, 128 * j - glo + 128)
    return wl, wh


def tt_layout():
    """Packed layout of the static band profile: [(gi, j, wl, wh, off)]."""
    out = []
    off = 0
    for gi in range(len(GROUPS)):
        for j in JSETS[gi]:
            wl, wh = band_window(gi, j)
            out.append((gi, j, wl, wh, off))
            off += wh - wl
    return out, off


TT_SEGS, TT_W = tt_layout()


def active_js(gi, a, b):
    """j-chunks of group gi whose band overlaps [a,b) (group-local)."""
    out = []
    for j in JSETS[gi]:
        wl, wh = band_window(gi, j)
        if wl < wh and wl < b and wh > a:
            out.append(j)
    return out


def build_nc():
    nc = bacc.Bacc(
        "TRN2",
        target_bir_lowering=False,
        debug=False,
        enable_asserts=True,
        num_devices=NCORES,
    )

    xT_d = nc.dram_tensor("xT", [D_PAD, R], F16, kind="ExternalInput")
    wp_d = nc.dram_tensor("wp", [D_PAD, H], F16, kind="ExternalInput")
    w1a_d = nc.dram_tensor("w1a", [H, H], F16, kind="ExternalInput")
    w1b_d = nc.dram_tensor("w1b", [H, H], F16, kind="ExternalInput")
    w2_d = nc.dram_tensor("w2", [H, H], F16, kind="ExternalInput")
    bp_d = nc.dram_tensor("bp", [H, 1], F32, kind="ExternalInput")
    b1_d = nc.dram_tensor("b1", [H, 1], F32, kind="ExternalInput")
    b1h_d = nc.dram_tensor("b1h", [H, 1], F32, kind="ExternalInput")
    b1f_d = nc.dram_tensor("b1f", [H, 1], F32, kind="ExternalInput")
    b2c2_d = nc.dram_tensor("b2c", [H, 1], F32, kind="ExternalInput")
    ident_d = nc.dram_tensor("ident", [P, P], F16, kind="ExternalInput")
    ttp_d = nc.dram_tensor("ttp", [P, TT_W], F16, kind="ExternalInput")
    spki_d = nc.dram_tensor("spki", [P, X], F16, kind="ExternalInput")
    ami_d = nc.dram_tensor("ami", [P, X], F16, kind="ExternalInput")
    bmi_d = nc.dram_tensor("bmi", [P, X], F16, kind="ExternalInput")
    cmi_d = nc.dram_tensor("cmi", [P, X], F16, kind="ExternalInput")
    spkj_d = nc.dram_tensor("spkj", [P, NCH], F32, kind="ExternalInput")
    namj_d = nc.dram_tensor("namj", [P, NCH], F32, kind="ExternalInput")
    nbmj_d = nc.dram_tensor("nbmj", [P, NCH], F32, kind="ExternalInput")
    ncmj_d = nc.dram_tensor("ncmj", [P, NCH], F32, kind="ExternalInput")
    hidx_d = nc.dram_tensor("hidx", [1, 2], U32, kind="ExternalInput")

    out_d = nc.dram_tensor("hT_out", [H, R], F32, kind="ExternalOutput")

    with tile.TileContext(nc) as tc:
        with (
            tc.tile_pool(name="consts", bufs=1) as cs,
            tc.tile_pool(name="sbuild", bufs=1) as sbp,
            tc.tile_pool(name="work", bufs=2) as wk,
            tc.tile_pool(name="states", bufs=1) as st,
            tc.tile_pool(name="ps", bufs=1, space="PSUM") as ps,
            tc.tile_pool(name="dram", bufs=1, space="DRAM") as dram,
        ):
            # ---------- critical path to AG0: xT/wp DMA -> proj -> h0 -------
            wp_r = cs.tile([P, ND, H], F16, tag="wp_r")
            nc.sync.dma_start(wp_r[:], wp_d[:].rearrange("(n p) m -> p n m", p=P))
            bp_c = cs.tile([H, 1], F32, tag="bp")
            nc.sync.dma_start(bp_c[:], bp_d[:])
            xT_s = cs.tile([P, ND, R], F16, tag="xT")
            xv = xT_d[:].rearrange("(n p) m -> p n m", p=P)
            for dd in range(ND):
                eng = (nc.sync, nc.scalar, nc.gpsimd)[dd % 3]
                eng.dma_start(xT_s[:, dd : dd + 1, :], xv[:, dd : dd + 1, :])
            hidx_sb = cs.tile([1, 2], U32, tag="hidx")
            nc.sync.dma_start(hidx_sb[:], hidx_d[:])
            regs_l = nc.alloc_registers("hl_reg")
            nc.regs_load(regs_l, hidx_sb[0:1, 0:1])
            hl_v = nc.snap(regs_l, donate=True)
            regs_r = nc.alloc_registers("hr_reg")
            nc.regs_load(regs_r, hidx_sb[0:1, 1:2])
            hr_v = nc.snap(regs_r, donate=True)

            h0_ps = ps.tile([P, R], F32, tag="k0")
            for d in range(ND):
                nc.tensor.matmul(h0_ps[:], wp_r[:, d, :], xT_s[:, d, :],
                                 start=(d == 0), stop=(d == ND - 1))
            hT = st.tile([P, R], F32, tag="hT")
            nc.scalar.activation(hT[:], h0_ps[:], AF.Identity, bias=bp_c[:],
                                 scale=1.0)
            h16 = st.tile([P, NY], F16, tag="h16")
            nc.vector.memset(h16[:, 0 : 4 * P], 0.0)
            nc.vector.memset(h16[:, 8 * P :], 0.0)
            nc.vector.tensor_copy(h16[:, 4 * P : 8 * P], hT[:])

            ag_in = dram.tile([P, R], F16, tag="ag_in")

            def do_ag(src, it):
                nc.sync.dma_start(ag_in[:], src)
                ag_out = dram.tile([NCORES * P, R], F16, tag=f"ago{it}",
                                   addr_space="Shared")
                nc.gpsimd.collective_compute(
                    "AllGather",
                    ALU.bypass,
                    replica_groups=[list(range(NCORES))],
                    ins=[ag_in[:].opt()],
                    outs=[ag_out[:].opt()],
                )
                return ag_out

            def fetch_halo(ag_out, dst):
                nc.scalar.dma_start(dst[:, 0 : 4 * P], ag_out[bass.ds(hl_v, P), :])
                nc.sync.dma_start(dst[:, 8 * P :], ag_out[bass.ds(hr_v, P), :])

            ag0 = do_ag(h16[:, 4 * P : 8 * P], 0)

            # ---------- remaining constants / tables ----------
            def load(dram_t, shape, name, dt=F16, eng=nc.scalar):
                t = cs.tile(shape, dt, tag=name)
                eng.dma_start(t[:], dram_t[:])
                return t

            w1a = load(w1a_d, [H, H], "w1a")
            w1b = load(w1b_d, [H, H], "w1b")
            w2 = load(w2_d, [H, H], "w2")
            b1_c = load(b1_d, [H, 1], "b1", F32)
            b1h_c = load(b1h_d, [H, 1], "b1h", F32)
            b1f_c = load(b1f_d, [H, 1], "b1f", F32)
            ident = load(ident_d, [P, P], "ident")
            ttp = load(ttp_d, [P, TT_W], "ttp", F16, nc.sync)
            spki = load(spki_d, [P, X], "spki", F16, nc.scalar)
            ami = load(ami_d, [P, X], "ami", F16, nc.scalar)
            bmi = load(bmi_d, [P, X], "bmi", F16, nc.scalar)
            cmi = load(cmi_d, [P, X], "cmi", F16, nc.scalar)
            spkj = load(spkj_d, [P, NCH], "spkj", F32, nc.scalar)
            namj = load(namj_d, [P, NCH], "namj", F32, nc.scalar)
            nbmj = load(nbmj_d, [P, NCH], "nbmj", F32, nc.scalar)
            ncmj = load(ncmj_d, [P, NCH], "ncmj", F32, nc.scalar)

            ones16 = cs.tile([P, 1], F16, tag="ones16")
            nc.vector.memset(ones16[:], 1.0)
            onesN = cs.tile([1, R], F16, tag="onesN")
            nc.vector.memset(onesN[:], 1.0)
            onesrow_f = cs.tile([1, P], F32, tag="onesrow_f")
            nc.vector.memset(onesrow_f[:], 1.0)
            onesrow = cs.tile([1, P], F32R, tag="onesrow")
            nc.vector.tensor_copy(onesrow[:], onesrow_f[:])
            b2c_d = cs.tile([H, 1], F32, tag="b2c")
            nc.sync.dma_start(b2c_d[:], b2c2_d[:])
            cb2h = cs.tile([H, 1], F32, tag="cb2h")
            nc.vector.tensor_scalar(cb2h[:], b2c_d[:], 0.5 * DT, None, ALU.mult)
            cb2f = cs.tile([H, 1], F32, tag="cb2f")
            nc.vector.tensor_scalar(cb2f[:], b2c_d[:], DT, None, ALU.mult)
            b2x6 = cs.tile([H, 1], F32, tag="b2x6")
            nc.vector.tensor_scalar(b2x6[:], b2c_d[:], 6.0, None, ALU.mult)

            with nc.allow_low_precision(reason="fp16 compute, f32 accumulate"):
                # ---------- banded S build (DVE + gpsimd) ----------
                s_raw = []     # per (gi, j): unnormalized fp16 tiles
                seg_off = {(gi, j): (wl, wh, off)
                           for gi, j, wl, wh, off in TT_SEGS}
                for gi, (glo, ghi) in enumerate(GROUPS):
                    gw = ghi - glo
                    row = {}
                    for j in JSETS[gi]:
                        t = sbp.tile([P, gw], F16, tag=f"sr{gi}_{j}")
                        wl, wh, off = seg_off[(gi, j)]
                        if wl > 0:
                            nc.gpsimd.memset(t[:, 0:wl], 0.0)
                        if wh < gw:
                            nc.gpsimd.memset(t[:, wh:gw], 0.0)
                        w = wh - wl
                        isl = slice(glo + wl, glo + wh)
                        jc = slice(j, j + 1)
                        da = wk.tile([P, w], F16, tag="da")
                        nc.scalar.activation(da[:], ami[:, isl], AF.Abs,
                                             bias=namj[:, jc], scale=1.0)
                        db = wk.tile([P, w], F16, tag="db")
                        nc.scalar.activation(db[:], bmi[:, isl], AF.Abs,
                                             bias=nbmj[:, jc], scale=1.0)
                        dc = wk.tile([P, w], F16, tag="dc")
                        nc.scalar.activation(dc[:], cmi[:, isl], AF.Abs,
                                             bias=ncmj[:, jc], scale=1.0)
                        e1 = wk.tile([P, w], F16, tag="e1")
                        nc.vector.tensor_tensor(e1[:], da[:], db[:], ALU.add)
                        e2 = wk.tile([P, w], F16, tag="e2")
                        nc.vector.tensor_tensor(e2[:], e1[:], dc[:], ALU.add)
                        uu = wk.tile([P, w], F16, tag="uu")
                        nc.vector.tensor_scalar(uu[:], e2[:], -1.0, A2,
                                                ALU.mult, ALU.add)
                        pm = wk.tile([P, w], F16, tag="pm")
                        nc.vector.tensor_scalar(pm[:], spki[:, isl],
                                                spkj[:, jc], A1, ALU.is_equal,
                                                ALU.mult)
                        qq = wk.tile([P, w], F16, tag="qq")
                        nc.vector.tensor_tensor(qq[:], uu[:], pm[:], ALU.max)
                        nc.vector.tensor_tensor(t[:, wl:wh],
                                                ttp[:, off : off + w], qq[:],
                                                ALU.mult)
                        row[j] = t
                    s_raw.append(row)

                # ---------- row sums -> fold 1/d into fp16 S ----------
                s_n = []
                for gi, (glo, ghi) in enumerate(GROUPS):
                    gw = ghi - glo
                    d_ps = ps.tile([P, 512], F32, tag="hn", bufs=2)
                    js = JSETS[gi]
                    for n, j in enumerate(js):
                        nc.tensor.matmul(d_ps[0:1, 0:gw], ones16[:],
                                         s_raw[gi][j][:],
                                         start=(n == 0), stop=(n == len(js) - 1))
                    dsum = wk.tile([1, gw], F32, tag="dsum")
                    nc.vector.tensor_scalar(dsum[:], d_ps[0:1, 0:gw], 1e-8,
                                            None, ALU.add)
                    rd = wk.tile([1, gw], F32R, tag="rd")
                    nc.vector.reciprocal(rd[:], dsum[:])
                    rdb_ps = ps.tile([P, 512], F32, tag="z1", bufs=2)
                    nc.tensor.matmul(rdb_ps[:, 0:gw], onesrow[:], rd[:])
                    row = {}
                    for j in JSETS[gi]:
                        t = cs.tile([P, gw], F16, tag=f"sn{gi}_{j}")
                        nc.vector.tensor_tensor(t[:], s_raw[gi][j][:],
                                                rdb_ps[:, 0:gw], ALU.mult)
                        row[j] = t
                    s_n.append(row)

                # ---------- h0 halo + initial row-form chunks ----------
                yr = st.tile([P, NCH, P], F16, tag="yr")

                def transpose_chunks(src16, chunks):
                    for c0 in range(0, len(chunks), 4):
                        grp = chunks[c0 : c0 + 4]
                        n = len(grp)
                        tp = ps.tile([P, 4, P], F16, tag="tr")
                        for k, ch in enumerate(grp):
                            nc.tensor.transpose(tp[:, k, :],
                                                src16[:, ch * P : (ch + 1) * P],
                                                ident[:])
                        nc.vector.tensor_copy(yr[:, grp[0] : grp[0] + n, :],
                                              tp[:, 0:n, :])

                transpose_chunks(h16, [4, 5, 6, 7])
                fetch_halo(ag0, h16)
                transpose_chunks(h16, [0, 1, 2, 3])
                transpose_chunks(h16, [8, 9, 10, 11])

                # ---------- RK4 loop ----------
                y16 = h16
                for step in range(N_STEPS):
                    for e in range(4):
                        lo, hi = EV_LO[e], EV_HI[e]
                        gidx = []
                        for gi, (glo, ghi) in enumerate(GROUPS):
                            a, b = max(lo, glo) - glo, min(hi, ghi) - glo
                            if a < b:
                                gidx.append((gi, a, b))
                        # software-pipelined per-group schedule:
                        # PE: S(g0), S(g1), z1(g0), S(g2), z1(g1), z2(g0),
                        #     z1(g2), z2(g1), z2(g2)
                        hns, hn16s, z1s, kps = {}, {}, {}, {}

                        def s_mm(gi, a, b):
                            js = active_js(gi, a, b)
                            hn = ps.tile([P, 512], F32, tag="hn", bufs=2)
                            for n, j in enumerate(js):
                                nc.tensor.matmul(hn[:, 0 : b - a], yr[:, j, :],
                                                 s_n[gi][j][:, a:b],
                                                 start=(n == 0),
                                                 stop=(n == len(js) - 1))
                            hns[gi] = hn
                            h16g = wk.tile([P, b - a], F16, tag="hn16")
                            nc.scalar.activation(h16g[:], hn[:, 0 : b - a],
                                                 AF.Copy, bias=0.0, scale=1.0)
                            hn16s[gi] = h16g

                        def z1_mm(gi, a, b):
                            glo = GROUPS[gi][0]
                            us = slice(glo + a + P, glo + b + P)
                            z1 = ps.tile([P, 512], F32, tag="z1", bufs=2)
                            nc.tensor.matmul(z1[:, 0 : b - a], w1a[:],
                                             y16[:, us], start=True, stop=False)
                            nc.tensor.matmul(z1[:, 0 : b - a], w1b[:],
                                             hn16s[gi][:], start=False,
                                             stop=True)
                            z1s[gi] = z1

                        # 1) all S matmuls (dense PE run); at e=0 put
                        # own-chunk work first to overlap the AllGather
                        if e == 0:
                            gidx = [gidx[1], gidx[0], gidx[2]]
                        hns = {}
                        for gi, a, b in gidx:
                            js = active_js(gi, a, b)
                            if e == 0:
                                js = ([j for j in js if 4 <= j <= 7]
                                      + [j for j in js if not 4 <= j <= 7])
                            hn = ps.tile([P, 512], F32, tag="hn", bufs=2)
                            for n, j in enumerate(js):
                                nc.tensor.matmul(hn[:, 0 : b - a], yr[:, j, :],
                                                 s_n[gi][j][:, a:b],
                                                 start=(n == 0),
                                                 stop=(n == len(js) - 1))
                            hns[gi] = hn
                        hn16s = {}
                        for gi, a, b in gidx:
                            h16g = wk.tile([P, b - a], F16, tag="hn16")
                            nc.scalar.activation(h16g[:], hns[gi][:, 0 : b - a],
                                                 AF.Copy, bias=0.0, scale=1.0)
                            hn16s[gi] = h16g
                        z1s = {}
                        for gi, a, b in gidx:
                            glo = GROUPS[gi][0]
                            us = slice(glo + a + P, glo + b + P)
                            z1 = ps.tile([P, 512], F32, tag="z1", bufs=2)
                            nc.tensor.matmul(z1[:, 0 : b - a], w1a[:],
                                             y16[:, us], start=True, stop=False)
                            nc.tensor.matmul(z1[:, 0 : b - a], w1b[:],
                                             hn16s[gi][:], start=False,
                                             stop=True)
                            z1s[gi] = z1
                        b1e = b1_c if e == 0 else (b1f_c if e == 3 else b1h_c)
                        kps = {}
                        for gi, a, b in gidx:
                            th = wk.tile([P, b - a], F16, tag="th")
                            nc.scalar.activation(th[:], z1s[gi][:, 0 : b - a],
                                                 AF.Tanh, bias=b1e[:],
                                                 scale=1.0)
                            kp = ps.tile([P, 512], F32, tag=f"k{gi}")
                            nc.tensor.matmul(kp[:, 0 : b - a], w2[:], th[:])
                            kps[gi] = (kp, a, b)

                        # RK4 accumulator (DVE; kp already includes b2)
                        own_pieces = []
                        for gi, (kp, a, b) in kps.items():
                            glo = GROUPS[gi][0]
                            oa, ob = max(OWN_LO, glo + a), min(OWN_HI, glo + b)
                            if oa < ob:
                                own_pieces.append(
                                    (kp[:, oa - glo - a : ob - glo - a],
                                     slice(oa - OWN_LO, ob - OWN_LO)))
                        if e == 0:
                            acc = st.tile([P, R], F32, tag="acc", bufs=2)
                            for kpsl, osl in own_pieces:
                                nc.vector.tensor_copy(acc[:, osl], kpsl)
                        elif e in (1, 2):
                            for kpsl, osl in own_pieces:
                                nc.vector.scalar_tensor_tensor(
                                    acc[:, osl], kpsl, 2.0, acc[:, osl],
                                    ALU.mult, ALU.add)
                        if e == 3:
                            last = step == N_STEPS - 1
                            hnew = st.tile([P, R], F32, tag=f"hT{step}")
                            for kpsl, osl in own_pieces:
                                a4 = wk.tile([P, osl.stop - osl.start], F32,
                                             tag="a4")
                                nc.vector.scalar_tensor_tensor(
                                    a4[:], kpsl, b2x6[:], acc[:, osl],
                                    ALU.add, ALU.add)
                                nc.vector.scalar_tensor_tensor(
                                    hnew[:, osl], a4[:], DT / 6.0, hT[:, osl],
                                    ALU.mult, ALU.add)
                            hT = hnew
                            if last:
                                nc.sync.dma_start(out_d[:], hT[:])
                                break
                            h16 = st.tile([P, NY], F16, tag=f"h16_{step}")
                            nc.vector.memset(h16[:, 0 : 4 * P], 0.0)
                            nc.vector.memset(h16[:, 8 * P :], 0.0)
                            nc.vector.tensor_copy(h16[:, 4 * P : 8 * P], hT[:])
                            ag = do_ag(h16[:, 4 * P : 8 * P], step + 1)
                            transpose_chunks(h16, [4, 5, 6, 7])
                            fetch_halo(ag, h16)
                            transpose_chunks(h16, [0, 1, 2, 3])
                            transpose_chunks(h16, [8, 9, 10, 11])
                            y16 = h16
                        else:
                            c = YCOEF[e]
                            nlo, nhi = EV_LO[e + 1], EV_HI[e + 1] + 2 * P
                            y16 = st.tile([P, NY], F16, tag="y16", bufs=2)
                            for gi, (kp, a, b) in kps.items():
                                glo = GROUPS[gi][0]
                                ys = slice(glo + a + P, glo + b + P)
                                nc.vector.scalar_tensor_tensor(
                                    y16[:, ys], kp[:, 0 : b - a], c,
                                    h16[:, ys], ALU.mult, ALU.add)
                            transpose_chunks(y16, list(range(nlo // P,
                                                             nhi // P)))

    nc.compile()
    return nc


def get_nc():
    global _CACHED_NC
    if _CACHED_NC is None:
        _CACHED_NC = build_nc()
    return _CACHED_NC


def prep_inputs(features, speaker_ids, modality_masks, Wp, bp, W1, b1, W2, b2):
    features = np.asarray(features, dtype=np.float32)
    spk = np.asarray(speaker_ids).astype(np.float32)
    mm = np.asarray(modality_masks, dtype=np.float32)
    Wp = np.asarray(Wp, dtype=np.float32)
    bp = np.asarray(bp, dtype=np.float32)
    W1 = np.asarray(W1, dtype=np.float32)
    b1 = np.asarray(b1, dtype=np.float32)
    W2 = np.asarray(W2, dtype=np.float32)
    b2 = np.asarray(b2, dtype=np.float32)

    wp_pad = np.zeros((D_PAD, H), dtype=np.float16)
    wp_pad[:D_IN] = Wp.astype(np.float16)
    msc = (mm * (A2 / 3.0)).astype(np.float32)
    ident = np.eye(P, dtype=np.float16)

    def repi(v, dt=np.float16):
        return np.ascontiguousarray(
            np.broadcast_to(v, (P, v.shape[0])), dtype=dt)

    in_maps = []
    for c in range(NCORES):
        own0 = c * R
        gx = own0 - 384 + np.arange(X)
        gxc = np.clip(gx, 0, B - 1)
        jv = own0 - 512 + np.arange(NY)
        jvalid = (jv >= 0) & (jv < B)
        jvc = np.clip(jv, 0, B - 1)
        # static band profile, packed over (gi, j) segments
        ttp = np.zeros((P, TT_W), dtype=np.float16)
        for gi, j, wl, wh, off in TT_SEGS:
            glo = GROUPS[gi][0]
            iv = gx[glo + wl : glo + wh][None, :]          # [1, w]
            jvv = jv[j * P : (j + 1) * P][:, None]         # [P, 1]
            val = np.exp(-BETA * np.abs(iv - jvv))
            val = np.where(iv == jvv, 1.0 / A1, val)       # diag fix
            jok = jvalid[j * P : (j + 1) * P][:, None]
            ttp[:, off : off + (wh - wl)] = np.where(jok, val, 0.0)
        xT = np.zeros((D_PAD, R), dtype=np.float16)
        xT[:D_IN] = features[own0 : own0 + R].T.astype(np.float16)
        hl_idx = P * (c - 1 if c > 0 else c)
        hr_idx = P * (c + 1 if c < NCORES - 1 else c)
        in_maps.append({
            "xT": xT,
            "wp": wp_pad,
            "w1a": W1[:H].astype(np.float16),
            "w1b": W1[H:].astype(np.float16),
            "w2": W2.astype(np.float16),
            "bp": bp.reshape(H, 1).astype(np.float32),
            "b1": b1.reshape(H, 1).astype(np.float32),
            "b1h": (b1 + 0.5 * DT * ((W1[:H] + W1[H:]).T @ b2)
                    ).reshape(H, 1).astype(np.float32),
            "b1f": (b1 + DT * ((W1[:H] + W1[H:]).T @ b2)
                    ).reshape(H, 1).astype(np.float32),
            "b2c": b2.reshape(H, 1).astype(np.float32),
            "ident": ident,
            "ttp": ttp,
            "spki": repi(spk[gxc].astype(np.float16)),
            "ami": repi(msc[gxc, 0].astype(np.float16)),
            "bmi": repi(msc[gxc, 1].astype(np.float16)),
            "cmi": repi(msc[gxc, 2].astype(np.float16)),
            "spkj": np.ascontiguousarray(
                spk[jvc].reshape(NCH, P).T, dtype=np.float32),
            "namj": np.ascontiguousarray(
                -msc[jvc, 0].reshape(NCH, P).T, dtype=np.float32),
            "nbmj": np.ascontiguousarray(
                -msc[jvc, 1].reshape(NCH, P).T, dtype=np.float32),
            "ncmj": np.ascontiguousarray(
                -msc[jvc, 2].reshape(NCH, P).T, dtype=np.float32),
            "hidx": np.array([[hl_idx, hr_idx]], dtype=np.uint32),
        })
    return in_maps


def kernel(features, speaker_ids, modality_masks, Wp, bp, W1, b1, W2, b2,
           _runner=None):
    in_maps = prep_inputs(features, speaker_ids, modality_masks,
                          Wp, bp, W1, b1, W2, b2)
    nc = get_nc()
    if _runner is not None:
        results = _runner(nc, in_maps)
    else:
        results = run_bass_kernel_spmd(nc, in_maps, list(range(NCORES))).results
    out = np.concatenate([results[c]["hT_out"].T for c in range(NCORES)], axis=0)
    return np.ascontiguousarray(out, dtype=np.float32)
